# revision 24
# baseline (speedup 1.0000x reference)
"""GAT message-passing kernel for Trainium2 (8 NeuronCores, data-parallel over batch).

Reference computation (per batch b):
    h_ope = feat_ope @ W_ope.T            [O, K]
    h_mac = feat_mac @ W_mac.T            [M, K]
    a_ope = h_ope @ alpha_ope             [O]
    a_mac = h_mac @ alpha_mac             [M]
    e     = leaky_relu(proc * (a_ope[:,None] + a_mac[None,:]), 0.2)
    alpha = softmax(where(proc==1, e, -9e10), axis=O) * proc
    out   = alpha.T @ h_ope + h_mac       [M, K]

Device decomposition (all per-element transcendentals eliminated):
  With s = a_ope[o] + a_mac[m] and proc in {0,1}:
    exp(leaky_relu(s)) = max(e^s, e^{0.2 s})
                       = p2[o] * max(p1[o]*q[m], q2[m])
  where p1 = e^{0.8 a_ope}, p2 = e^{0.2 a_ope}, q = e^{a_mac}, q2 = e^{0.2 a_mac}
  (all computed on host in f64).  The p2[o] row-scale is folded into a
  host-premultiplied featA[o, 0:64] = feat_ope[o,:]*p2[o], featA[o,64] = p2[o],
  so the softmax denominator Z falls out of the same PE accumulation:
    acc[0:64, m] = sum_o featA[o,0:64] * w[o,m]   (= (alpha_unnorm.T @ h_ope) @ ...)
    acc[64,   m] = sum_o p2[o] * w[o,m] = Z[m]
  with w = max(p1*q outer-product, q2_bcast) * proc.
  Final: out[m,k] = (acc[0:64].T @ W_ope.T)[m,k] / Z[m] + (feat_mac @ W_mac.T)[m,k]

Per-core work: 8 batches; per batch 32 o-tiles of 125 partitions (32*125=4000),
processed in 8 chunks of 4 tiles.
"""

import numpy as np

import concourse.bacc as bacc
import concourse.mybir as mybir
from concourse import tile
from concourse.bass_utils import run_bass_kernel_spmd

F32 = mybir.dt.float32

B, O, M, D = 64, 4000, 128, 64
NCORES = 8
BPC = B // NCORES          # batches per core
P = 125                    # o rows per tile (partition dim)
TPB = O // P               # 32 tiles per batch
CH = 4                     # tiles per chunk
NCH = TPB // CH            # 8 chunks per batch
CHROWS = P * CH            # 500 o rows per chunk

_CACHE = {}


def _build_nc(repeat=1):
    nc = bacc.Bacc(None, target_bir_lowering=False)

    # featA/proc are host-pre-tiled: [b, chunk, p, j, d] so each chunk DMA is
    # one contiguous 1-2KB read per partition.
    featA = nc.dram_tensor("featA", [BPC, NCH, P, CH, D + 1], F32, kind="ExternalInput")
    proc = nc.dram_tensor("proc", [BPC, NCH, P, CH, M], mybir.dt.uint8,
                          kind="ExternalInput")
    fmT = nc.dram_tensor("fmT", [BPC, D, M], F32, kind="ExternalInput")
    # rowpack = [p1 (O) | q (M) | q2x4 (CH*M)] per batch
    ROWLEN = O + M + CH * M
    rowpack = nc.dram_tensor("rowpack", [BPC, ROWLEN], F32, kind="ExternalInput")
    wot = nc.dram_tensor("wot", [D, D], F32, kind="ExternalInput")
    wmt = nc.dram_tensor("wmt", [D, D], F32, kind="ExternalInput")
    out = nc.dram_tensor("out", [BPC, M, D], F32, kind="ExternalOutput")

    with tile.TileContext(nc) as tc:
        with (
            tc.tile_pool(name="const", bufs=1) as cpool,
            tc.tile_pool(name="rows", bufs=2) as rpool,
            tc.tile_pool(name="feat", bufs=5) as fpool,
            tc.tile_pool(name="big", bufs=5) as bpool,
            tc.tile_pool(name="fin", bufs=2) as xpool,
            tc.tile_pool(name="ps_e", bufs=4, space="PSUM") as ps_e,
            tc.tile_pool(name="ps_u", bufs=2, space="PSUM") as ps_u,
            tc.tile_pool(name="ps_b", bufs=1, space="PSUM") as ps_b,
            tc.tile_pool(name="ps_f", bufs=1, space="PSUM") as ps_f,
        ):
            ones_r = cpool.tile([1, P], F32)
            nc.vector.memset(ones_r[:], 1.0)
            one1 = cpool.tile([1, 1], F32)
            nc.vector.memset(one1[:], 1.0)
            wot_s = cpool.tile([D, D], F32)
            nc.sync.dma_start(wot_s[:], wot[:])
            wmt_s = cpool.tile([D, D], F32)
            nc.sync.dma_start(wmt_s[:], wmt[:])

            for b in [b for _ in range(repeat) for b in range(BPC)]:
                row_s = rpool.tile([1, ROWLEN], F32)
                nc.sync.dma_start(row_s[:], rowpack[b : b + 1, :])
                fmT_s = rpool.tile([D, M], F32)
                nc.sync.dma_start(fmT_s[:], fmT[b])

                # broadcast q2 (tiled x4) across 125 partitions via PE outer product
                q2b_ps = ps_b.tile([P, CH * M], F32)
                nc.tensor.matmul(q2b_ps[:], ones_r[:], row_s[0:1, O + M : ROWLEN])
                q2b_s = rpool.tile([P, CH, M], F32)
                nc.scalar.copy(q2b_s[:], q2b_ps[:].rearrange("p (j m) -> p j m", j=CH))

                u2t_ps = ps_u.tile([D + 1, M], F32)

                for c in range(NCH):
                    fa = fpool.tile([P, CH, D + 1], F32)
                    nc.sync.dma_start(fa[:], featA[b, c])
                    pr8 = fpool.tile([P, CH, M], mybir.dt.uint8, tag="pr8")
                    nc.sync.dma_start(pr8[:], proc[b, c])
                    pr = bpool.tile([P, CH, M], F32)
                    nc.scalar.copy(pr[:], pr8[:])

                    e_ps = ps_e.tile([P, CH, M], F32)
                    for j in range(CH):
                        t = c * CH + j
                        nc.tensor.matmul(
                            e_ps[:, j, :],
                            row_s[0:1, t * P : (t + 1) * P],
                            row_s[0:1, O : O + M],
                        )

                    m1 = bpool.tile([P, CH, M], F32)
                    nc.vector.tensor_tensor(
                        m1[:], e_ps[:], q2b_s[:], mybir.AluOpType.max
                    )
                    w = bpool.tile([P, CH, M], F32)
                    nc.gpsimd.tensor_tensor(w[:], m1[:], pr[:], mybir.AluOpType.mult)

                    for j in range(CH):
                        t = c * CH + j
                        nc.tensor.matmul(
                            u2t_ps[:],
                            fa[:, j, :],
                            w[:, j, :],
                            start=(t == 0),
                            stop=(t == TPB - 1),
                        )

                # ---- per-batch finals ----
                u2t_s = xpool.tile([D + 1, M], F32)
                nc.scalar.copy(u2t_s[:], u2t_ps[:])

                rz_row = xpool.tile([1, M], F32)
                nc.vector.reciprocal(rz_row[:], u2t_s[D : D + 1, :])

                # one PSUM bank holds out3 | h_mac | rz side by side
                fin_ps = ps_f.tile([M, 2 * D + 1], F32, tag="fin")
                nc.tensor.matmul(fin_ps[:, 2 * D : 2 * D + 1], rz_row[:], one1[:])
                rz_s = xpool.tile([M, 1], F32)
                nc.scalar.copy(rz_s[:], fin_ps[:, 2 * D : 2 * D + 1])

                nc.tensor.matmul(fin_ps[:, 0:D], u2t_s[0:D, :], wot_s[:])
                nc.tensor.matmul(fin_ps[:, D : 2 * D], fmT_s[:], wmt_s[:])

                hm_s = xpool.tile([M, D], F32)
                nc.scalar.copy(hm_s[:], fin_ps[:, D : 2 * D])
                outf = xpool.tile([M, D], F32)
                nc.vector.scalar_tensor_tensor(
                    outf[:],
                    fin_ps[:, 0:D],
                    rz_s[:],
                    hm_s[:],
                    mybir.AluOpType.mult,
                    mybir.AluOpType.add,
                )
                nc.sync.dma_start(out[b], outf[:])

    nc.finalize()
    return nc


def _host_precompute(curr_proc_batch, batch_idxes, feat_ope, feat_mac, W_ope, W_mac,
                     alpha_ope, alpha_mac):
    bi = np.asarray(batch_idxes)
    if np.array_equal(bi, np.arange(B)):
        proc = np.ascontiguousarray(curr_proc_batch, dtype=np.float32)
    else:
        proc = np.ascontiguousarray(curr_proc_batch[bi.astype(np.int64)],
                                    dtype=np.float32)

    fo = np.asarray(feat_ope, dtype=np.float32)
    fm = np.asarray(feat_mac, dtype=np.float32)
    Wo = np.asarray(W_ope, dtype=np.float64)
    Wm = np.asarray(W_mac, dtype=np.float64)

    v_ope = Wo.T @ np.asarray(alpha_ope, dtype=np.float64)
    v_mac = Wm.T @ np.asarray(alpha_mac, dtype=np.float64)
    a_ope = fo.astype(np.float64) @ v_ope          # [B, O]
    a_mac = fm.astype(np.float64) @ v_mac          # [B, M]

    p1 = np.exp(0.8 * a_ope).astype(np.float32)
    p2 = np.exp(0.2 * a_ope).astype(np.float32)
    q = np.exp(a_mac).astype(np.float32)
    q2 = np.exp(0.2 * a_mac).astype(np.float32)

    featA = np.empty((B, O, D + 1), np.float32)
    featA[:, :, :D] = fo * p2[:, :, None]
    featA[:, :, D] = p2
    # pre-tile: [b, o=(c j p), d] -> [b, c, p, j, d] so chunk DMAs are contiguous
    featA = np.ascontiguousarray(
        featA.reshape(B, NCH, CH, P, D + 1).transpose(0, 1, 3, 2, 4))
    proc = np.ascontiguousarray(
        proc.astype(np.uint8).reshape(B, NCH, CH, P, M).transpose(0, 1, 3, 2, 4))

    rowpack = np.concatenate([p1, q, np.tile(q2, (1, CH))], axis=1)  # [B, O+M+CH*M]
    fmT = np.ascontiguousarray(fm.transpose(0, 2, 1))
    wot = np.ascontiguousarray(Wo.T).astype(np.float32)
    wmt = np.ascontiguousarray(Wm.T).astype(np.float32)
    return proc, featA, fmT, rowpack, wot, wmt


def kernel(curr_proc_batch, batch_idxes, feat_ope, feat_mac, W_ope, W_mac,
           alpha_ope, alpha_mac, _run_kwargs=None):
    proc, featA, fmT, rowpack, wot, wmt = _host_precompute(
        curr_proc_batch, batch_idxes, feat_ope, feat_mac, W_ope, W_mac,
        alpha_ope, alpha_mac)

    if "nc" not in _CACHE:
        _CACHE["nc"] = _build_nc()
    nc = _CACHE["nc"]

    in_maps = []
    for c in range(NCORES):
        s = slice(c * BPC, (c + 1) * BPC)
        in_maps.append({
            "featA": featA[s],
            "proc": proc[s],
            "fmT": fmT[s],
            "rowpack": rowpack[s],
            "wot": wot,
            "wmt": wmt,
        })

    kw = _run_kwargs or {}
    res = run_bass_kernel_spmd(nc, in_maps, list(range(NCORES)), **kw)
    _CACHE["last_results"] = res
    out = np.concatenate([res.results[c]["out"] for c in range(NCORES)], axis=0)
    return out


# revision 28
# speedup vs baseline: 1.6730x; 1.6730x over previous
"""GAT message-passing kernel for Trainium2 (8 NeuronCores, data-parallel over batch).

Reference computation (per batch b):
    h_ope = feat_ope @ W_ope.T            [O, K]
    h_mac = feat_mac @ W_mac.T            [M, K]
    a_ope = h_ope @ alpha_ope             [O]
    a_mac = h_mac @ alpha_mac             [M]
    e     = leaky_relu(proc * (a_ope[:,None] + a_mac[None,:]), 0.2)
    alpha = softmax(where(proc==1, e, -9e10), axis=O) * proc
    out   = alpha.T @ h_ope + h_mac       [M, K]

Device decomposition (all per-element transcendentals eliminated):
  With s = a_ope[o] + a_mac[m] and proc in {0,1}:
    exp(leaky_relu(s)) = max(e^s, e^{0.2 s})
                       = p2[o] * max(p1[o]*q[m], q2[m])
  where p1 = e^{0.8 a_ope}, p2 = e^{0.2 a_ope}, q = e^{a_mac}, q2 = e^{0.2 a_mac}
  (all computed on host in f64).  The p2[o] row-scale is folded into a
  host-premultiplied featA[o, 0:64] = feat_ope[o,:]*p2[o], featA[o,64] = p2[o],
  so the softmax denominator Z falls out of the same PE accumulation:
    acc[0:64, m] = sum_o featA[o,0:64] * w[o,m]   (= (alpha_unnorm.T @ h_ope) @ ...)
    acc[64,   m] = sum_o p2[o] * w[o,m] = Z[m]
  with w = max(p1*q outer-product, q2_bcast) * proc.
  Final: out[m,k] = (acc[0:64].T @ W_ope.T)[m,k] / Z[m] + (feat_mac @ W_mac.T)[m,k]

Per-core work: 8 batches; per batch 32 o-tiles of 125 partitions (32*125=4000),
processed in 8 chunks of 4 tiles.
"""

import numpy as np

import concourse.bacc as bacc
import concourse.mybir as mybir
from concourse import tile
from concourse.bass_utils import run_bass_kernel_spmd

F32 = mybir.dt.float32

B, O, M, D = 64, 4000, 128, 64
NCORES = 8
BPC = B // NCORES          # batches per core
P = 125                    # o rows per tile (partition dim)
TPB = O // P               # 32 tiles per batch
CH = 4                     # tiles per chunk
NCH = TPB // CH            # 8 chunks per batch
CHROWS = P * CH            # 500 o rows per chunk

_CACHE = {}


def _build_nc(repeat=1):
    nc = bacc.Bacc(None, target_bir_lowering=False)

    # blob: host-pre-tiled per (batch, chunk, partition): 1040B featA f32 bytes
    # followed by 512B proc u8 — one DMA per chunk.
    BLOBW = CH * (D + 1) * 4 + CH * M
    blob = nc.dram_tensor("blob", [BPC, NCH, P, BLOBW], mybir.dt.uint8,
                          kind="ExternalInput")
    fmT = nc.dram_tensor("fmT", [BPC, D, M], F32, kind="ExternalInput")
    # rowsA: all per-batch row data, 4 partitions, DMA'd once at kernel start.
    # cols [0, 8000): p1 in [j, b*1000 + c*125 + p] layout
    # cols [8000, 12096): block-diagonal q per batch [4, 512]
    # cols [12096, 16192): row 0 = q2 tiled x4 per batch
    P1W = BPC * NCH * P            # 8000
    QDW = BPC * CH * M             # 4096
    ROWSW = P1W + 2 * QDW          # 16192
    rowsA = nc.dram_tensor("rowsA", [CH, ROWSW], F32, kind="ExternalInput")
    wot = nc.dram_tensor("wot", [D, D], F32, kind="ExternalInput")
    wmt = nc.dram_tensor("wmt", [D, D], F32, kind="ExternalInput")
    out = nc.dram_tensor("out", [BPC, M, D], F32, kind="ExternalOutput")
    F32R = mybir.dt.float32r

    with tile.TileContext(nc) as tc:
        with (
            tc.tile_pool(name="const", bufs=1) as cpool,
            tc.tile_pool(name="rows", bufs=2) as rpool,
            tc.tile_pool(name="feat", bufs=5) as fpool,
            tc.tile_pool(name="big", bufs=5) as bpool,
            tc.tile_pool(name="fin", bufs=2) as xpool,
            tc.tile_pool(name="ps_e", bufs=4, space="PSUM") as ps_e,
            tc.tile_pool(name="ps_u", bufs=2, space="PSUM") as ps_u,
            tc.tile_pool(name="ps_b", bufs=1, space="PSUM") as ps_b,
            tc.tile_pool(name="ps_f", bufs=1, space="PSUM") as ps_f,
        ):
            ones_r = cpool.tile([1, P], F32)
            nc.vector.memset(ones_r[:], 1.0)
            one1 = cpool.tile([1, 1], F32)
            nc.vector.memset(one1[:], 1.0)
            wot_s = cpool.tile([D, D], F32)
            nc.sync.dma_start(wot_s[:], wot[:])
            wmt_s = cpool.tile([D, D], F32)
            nc.sync.dma_start(wmt_s[:], wmt[:])
            rows_s = cpool.tile([CH, ROWSW], F32)
            nc.sync.dma_start(rows_s[:], rowsA[:])

            for b in [b for _ in range(repeat) for b in range(BPC)]:
                fmT_s = rpool.tile([D, M], F32)
                nc.sync.dma_start(fmT_s[:], fmT[b])

                # broadcast q2 (tiled x4) across 125 partitions via PE outer product
                q2off = P1W + QDW + b * CH * M
                q2b_ps = ps_b.tile([P, CH * M], F32)
                nc.tensor.matmul(q2b_ps[:], ones_r[:],
                                 rows_s[0:1, q2off : q2off + CH * M])
                q2b_s = rpool.tile([P, CH, M], F32)
                nc.scalar.copy(q2b_s[:], q2b_ps[:].rearrange("p (j m) -> p j m", j=CH))

                u2t_ps = ps_u.tile([D + 1, M], F32)

                qd = rows_s[0:CH, P1W + b * CH * M : P1W + (b + 1) * CH * M]
                for c in range(NCH):
                    bl = fpool.tile([P, BLOBW], mybir.dt.uint8)
                    nc.sync.dma_start(bl[:], blob[b, c])
                    fa = bl[:, 0 : CH * (D + 1) * 4].bitcast(F32).rearrange(
                        "p (j d) -> p j d", j=CH)
                    pr8 = bl[:, CH * (D + 1) * 4 : BLOBW].rearrange(
                        "p (j m) -> p j m", j=CH)
                    pr = bpool.tile([P, CH, M], F32)
                    nc.scalar.copy(pr[:], pr8)

                    # all 4 outer products of the chunk in one f32r matmul:
                    # lhsT = p1 rows [4, 125], rhs = block-diag q [4, 512]
                    p1off = b * NCH * P + c * P
                    e_ps = ps_e.tile([P, CH, M], F32)
                    nc.tensor.matmul(
                        e_ps[:].rearrange("p j m -> p (j m)"),
                        rows_s[0:CH, p1off : p1off + P].bitcast(F32R),
                        qd.bitcast(F32R),
                    )

                    m1 = bpool.tile([P, CH, M], F32)
                    nc.vector.tensor_tensor(
                        m1[:], e_ps[:], q2b_s[:], mybir.AluOpType.max
                    )
                    w = bpool.tile([P, CH, M], F32)
                    nc.gpsimd.tensor_tensor(w[:], m1[:], pr[:], mybir.AluOpType.mult)

                    for j in range(CH):
                        t = c * CH + j
                        nc.tensor.matmul(
                            u2t_ps[:],
                            fa[:, j, :],
                            w[:, j, :],
                            start=(t == 0),
                            stop=(t == TPB - 1),
                        )

                # ---- per-batch finals ----
                u2t_s = xpool.tile([D + 1, M], F32)
                nc.scalar.copy(u2t_s[:], u2t_ps[:])

                rz_row = xpool.tile([1, M], F32)
                nc.vector.reciprocal(rz_row[:], u2t_s[D : D + 1, :])

                # one PSUM bank holds out3 | h_mac | rz side by side
                fin_ps = ps_f.tile([M, 2 * D + 1], F32, tag="fin")
                nc.tensor.matmul(fin_ps[:, 2 * D : 2 * D + 1], rz_row[:], one1[:])
                rz_s = xpool.tile([M, 1], F32)
                nc.scalar.copy(rz_s[:], fin_ps[:, 2 * D : 2 * D + 1])

                nc.tensor.matmul(fin_ps[:, 0:D], u2t_s[0:D, :], wot_s[:])
                nc.tensor.matmul(fin_ps[:, D : 2 * D], fmT_s[:], wmt_s[:])

                hm_s = xpool.tile([M, D], F32)
                nc.scalar.copy(hm_s[:], fin_ps[:, D : 2 * D])
                outf = xpool.tile([M, D], F32)
                nc.vector.scalar_tensor_tensor(
                    outf[:],
                    fin_ps[:, 0:D],
                    rz_s[:],
                    hm_s[:],
                    mybir.AluOpType.mult,
                    mybir.AluOpType.add,
                )
                nc.sync.dma_start(out[b], outf[:])

    nc.finalize()
    return nc


def _host_precompute(curr_proc_batch, batch_idxes, feat_ope, feat_mac, W_ope, W_mac,
                     alpha_ope, alpha_mac):
    bi = np.asarray(batch_idxes)
    if np.array_equal(bi, np.arange(B)):
        proc = np.ascontiguousarray(curr_proc_batch, dtype=np.float32)
    else:
        proc = np.ascontiguousarray(curr_proc_batch[bi.astype(np.int64)],
                                    dtype=np.float32)

    fo = np.asarray(feat_ope, dtype=np.float32)
    fm = np.asarray(feat_mac, dtype=np.float32)
    Wo = np.asarray(W_ope, dtype=np.float64)
    Wm = np.asarray(W_mac, dtype=np.float64)

    v_ope = Wo.T @ np.asarray(alpha_ope, dtype=np.float64)
    v_mac = Wm.T @ np.asarray(alpha_mac, dtype=np.float64)
    a_ope = fo.astype(np.float64) @ v_ope          # [B, O]
    a_mac = fm.astype(np.float64) @ v_mac          # [B, M]

    p1 = np.exp(0.8 * a_ope).astype(np.float32)
    p2 = np.exp(0.2 * a_ope).astype(np.float32)
    q = np.exp(a_mac).astype(np.float32)
    q2 = np.exp(0.2 * a_mac).astype(np.float32)

    featA = np.empty((B, O, D + 1), np.float32)
    featA[:, :, :D] = fo * p2[:, :, None]
    featA[:, :, D] = p2
    # pre-tile to [b, c, p, j, d] and byte-pack featA + u8 proc into one blob
    featA_t = np.ascontiguousarray(
        featA.reshape(B, NCH, CH, P, D + 1).transpose(0, 1, 3, 2, 4))
    proc_t = np.ascontiguousarray(
        proc.astype(np.uint8).reshape(B, NCH, CH, P, M).transpose(0, 1, 3, 2, 4))
    fa_bytes = featA_t.view(np.uint8).reshape(B, NCH, P, CH * (D + 1) * 4)
    pr_bytes = proc_t.reshape(B, NCH, P, CH * M)
    blob = np.concatenate([fa_bytes, pr_bytes], axis=-1)   # [B, NCH, P, BLOBW]

    # rowsA per core: p1 rows | block-diag q | q2x4 row
    P1W = BPC * NCH * P
    QDW = BPC * CH * M
    rowsA = np.zeros((NCORES, CH, P1W + 2 * QDW), np.float32)
    for core in range(NCORES):
        for bl in range(BPC):
            g = core * BPC + bl
            # p1_4[j, c*P + p] = p1[g, (c*CH + j)*P + p]
            p14 = p1[g].reshape(NCH, CH, P).transpose(1, 0, 2).reshape(CH, NCH * P)
            rowsA[core, :, bl * NCH * P : (bl + 1) * NCH * P] = p14
            for j in range(CH):
                rowsA[core, j, P1W + bl * CH * M + j * M : P1W + bl * CH * M + (j + 1) * M] = q[g]
            rowsA[core, 0, P1W + QDW + bl * CH * M : P1W + QDW + (bl + 1) * CH * M] = \
                np.tile(q2[g], CH)

    fmT = np.ascontiguousarray(fm.transpose(0, 2, 1))
    wot = np.ascontiguousarray(Wo.T).astype(np.float32)
    wmt = np.ascontiguousarray(Wm.T).astype(np.float32)
    return blob, rowsA, fmT, wot, wmt


def kernel(curr_proc_batch, batch_idxes, feat_ope, feat_mac, W_ope, W_mac,
           alpha_ope, alpha_mac, _run_kwargs=None):
    blob, rowsA, fmT, wot, wmt = _host_precompute(
        curr_proc_batch, batch_idxes, feat_ope, feat_mac, W_ope, W_mac,
        alpha_ope, alpha_mac)

    if "nc" not in _CACHE:
        _CACHE["nc"] = _build_nc()
    nc = _CACHE["nc"]

    in_maps = []
    for c in range(NCORES):
        s = slice(c * BPC, (c + 1) * BPC)
        in_maps.append({
            "blob": blob[s],
            "rowsA": rowsA[c],
            "fmT": fmT[s],
            "wot": wot,
            "wmt": wmt,
        })

    kw = _run_kwargs or {}
    res = run_bass_kernel_spmd(nc, in_maps, list(range(NCORES)), **kw)
    _CACHE["last_results"] = res
    out = np.concatenate([res.results[c]["out"] for c in range(NCORES)], axis=0)
    return out


# revision 45
# speedup vs baseline: 2.7067x; 1.6178x over previous
"""GAT message-passing kernel for Trainium2 (8 NeuronCores, data-parallel over batch).

Reference computation (per batch b):
    h_ope = feat_ope @ W_ope.T            [O, K]
    h_mac = feat_mac @ W_mac.T            [M, K]
    a_ope = h_ope @ alpha_ope             [O]
    a_mac = h_mac @ alpha_mac             [M]
    e     = leaky_relu(proc * (a_ope[:,None] + a_mac[None,:]), 0.2)
    alpha = softmax(where(proc==1, e, -9e10), axis=O) * proc
    out   = alpha.T @ h_ope + h_mac       [M, K]

Device decomposition (all per-element transcendentals eliminated):
  With s = a_ope[o] + a_mac[m] and proc in {0,1}:
    exp(leaky_relu(s)) = max(e^s, e^{0.2 s})
                       = p2[o] * max(p1[o]*q[m], q2[m])
  where p1 = e^{0.8 a_ope}, p2 = e^{0.2 a_ope}, q = e^{a_mac}, q2 = e^{0.2 a_mac}
  (all computed on host in f64).  The p2[o] row-scale is folded into a
  host-premultiplied featA[o, 0:64] = feat_ope[o,:]*p2[o], featA[o,64] = p2[o],
  so the softmax denominator Z falls out of the same PE accumulation:
    acc[0:64, m] = sum_o featA[o,0:64] * w[o,m]   (= (alpha_unnorm.T @ h_ope) @ ...)
    acc[64,   m] = sum_o p2[o] * w[o,m] = Z[m]
  with w = max(p1*q outer-product, q2_bcast) * proc.
  Final: out[m,k] = (acc[0:64].T @ W_ope.T)[m,k] / Z[m] + (feat_mac @ W_mac.T)[m,k]

Per-core work: 8 batches; per batch 32 o-tiles of 125 partitions (32*125=4000),
processed in 8 chunks of 4 tiles.
"""

import numpy as np

import concourse.bacc as bacc
import concourse.mybir as mybir
from concourse import tile
from concourse.bass_utils import run_bass_kernel_spmd

F32 = mybir.dt.float32

B, O, M, D = 64, 4000, 128, 64
NCORES = 8
BPC = B // NCORES          # batches per core
P = 125                    # o rows per tile (partition dim)
TPB = O // P               # 32 tiles per batch
CH = 4                     # tiles per chunk
NCH = TPB // CH            # 8 chunks per batch
CHROWS = P * CH            # 500 o rows per chunk

_CACHE = {}


def _build_nc(repeat=1):
    nc = bacc.Bacc(None, target_bir_lowering=False)

    # blob: host-pre-tiled per (batch, chunk, partition): 1040B featA f32 bytes
    # followed by 512B proc u8 — one DMA per chunk.
    BLOBW = CH * (D + 1) * 4 + CH * M
    blob = nc.dram_tensor("blob", [BPC, NCH, P, BLOBW], mybir.dt.uint8,
                          kind="ExternalInput")
    fmT = nc.dram_tensor("fmT", [BPC, D, M], F32, kind="ExternalInput")
    # rowsA: per-batch row data on 4 partitions, one strip per batch:
    # strip b = [p1 (NCH*P) | block-diag q (CH*M) | q2x4 on row 0 (CH*M)]
    STRW = NCH * P + 2 * CH * M    # 2024
    ROWSW = BPC * STRW
    F32R = mybir.dt.float32r
    rowsA = nc.dram_tensor("rowsA", [BPC, CH, STRW], F32R, kind="ExternalInput")
    q2r = nc.dram_tensor("q2r", [BPC, CH * M], F32, kind="ExternalInput")
    wot = nc.dram_tensor("wot", [D, D], F32, kind="ExternalInput")
    wmt = nc.dram_tensor("wmt", [D, D], F32, kind="ExternalInput")
    out = nc.dram_tensor("out", [BPC, M, D], F32, kind="ExternalOutput")

    with tile.TileContext(nc) as tc:
        with (
            tc.tile_pool(name="const", bufs=1) as cpool,
            tc.tile_pool(name="rows", bufs=2) as rpool,
            tc.tile_pool(name="feat", bufs=5) as fpool,
            tc.tile_pool(name="big", bufs=5) as bpool,
            tc.tile_pool(name="fin", bufs=2) as xpool,
            tc.tile_pool(name="ps_e", bufs=4, space="PSUM") as ps_e,
            tc.tile_pool(name="ps_u", bufs=2, space="PSUM") as ps_u,
            tc.tile_pool(name="ps_b", bufs=1, space="PSUM") as ps_b,
            tc.tile_pool(name="ps_f", bufs=1, space="PSUM") as ps_f,
        ):
            ones_r = cpool.tile([1, P], F32)
            nc.vector.memset(ones_r[:], 1.0)
            one1 = cpool.tile([1, 1], F32)
            nc.vector.memset(one1[:], 1.0)
            wot_s = cpool.tile([D, D], F32)
            nc.sync.dma_start(wot_s[:], wot[:])
            wmt_s = cpool.tile([D, D], F32)
            nc.sync.dma_start(wmt_s[:], wmt[:])
            rows_s = cpool.tile([CH, ROWSW], F32R)
            for bb in range(BPC):
                nc.sync.dma_start(
                    rows_s[:, bb * STRW : (bb + 1) * STRW], rowsA[bb])

            for b in [b for _ in range(repeat) for b in range(BPC)]:
                fmT_s = rpool.tile([D, M], F32)
                nc.sync.dma_start(fmT_s[:], fmT[b])

                # broadcast q2 (tiled x4) across 125 partitions via PE outer product
                q2_s = rpool.tile([1, CH * M], F32)
                nc.sync.dma_start(q2_s[:], q2r[b : b + 1, :])
                q2b_ps = ps_b.tile([P, CH * M], F32)
                nc.tensor.matmul(q2b_ps[:], ones_r[:], q2_s[:])
                q2b_s = rpool.tile([P, CH, M], F32)
                nc.scalar.copy(q2b_s[:], q2b_ps[:].rearrange("p (j m) -> p j m", j=CH))

                u2t_ps = ps_u.tile([D + 1, M], F32)

                qd = rows_s[0:CH, b * STRW + NCH * P : b * STRW + NCH * P + CH * M]
                for c in range(NCH):
                    bl = fpool.tile([P, BLOBW], mybir.dt.uint8)
                    nc.sync.dma_start(bl[:], blob[b, c])
                    fa = bl[:, 0 : CH * (D + 1) * 4].bitcast(F32).rearrange(
                        "p (j d) -> p j d", j=CH)
                    pr8 = bl[:, CH * (D + 1) * 4 : BLOBW].rearrange(
                        "p (j m) -> p j m", j=CH)
                    pr = bpool.tile([P, CH, M], F32)
                    nc.scalar.copy(pr[:], pr8)

                    # all 4 outer products of the chunk in one f32r matmul:
                    # lhsT = p1 rows [4, 125], rhs = block-diag q [4, 512]
                    p1off = b * STRW + c * P
                    e_ps = ps_e.tile([P, CH, M], F32)
                    nc.tensor.matmul(
                        e_ps[:].rearrange("p j m -> p (j m)"),
                        rows_s[0:CH, p1off : p1off + P],
                        qd,
                    )

                    m1 = bpool.tile([P, CH, M], F32)
                    nc.vector.tensor_tensor(
                        m1[:], e_ps[:], q2b_s[:], mybir.AluOpType.max
                    )
                    w = bpool.tile([P, CH, M], F32)
                    nc.gpsimd.tensor_tensor(w[:], m1[:], pr[:], mybir.AluOpType.mult)

                    for j in range(CH):
                        t = c * CH + j
                        nc.tensor.matmul(
                            u2t_ps[:],
                            fa[:, j, :],
                            w[:, j, :],
                            start=(t == 0),
                            stop=(t == TPB - 1),
                        )

                # ---- per-batch finals ----
                u2t_s = xpool.tile([D + 1, M], F32)
                nc.scalar.copy(u2t_s[:], u2t_ps[:])

                rz_row = xpool.tile([1, M], F32)
                nc.vector.reciprocal(rz_row[:], u2t_s[D : D + 1, :])

                # one PSUM bank holds out3 | h_mac | rz side by side
                fin_ps = ps_f.tile([M, 2 * D + 1], F32, tag="fin")
                nc.tensor.matmul(fin_ps[:, 2 * D : 2 * D + 1], rz_row[:], one1[:])
                rz_s = xpool.tile([M, 1], F32)
                nc.scalar.copy(rz_s[:], fin_ps[:, 2 * D : 2 * D + 1])

                nc.tensor.matmul(fin_ps[:, 0:D], u2t_s[0:D, :], wot_s[:])
                nc.tensor.matmul(fin_ps[:, D : 2 * D], fmT_s[:], wmt_s[:])

                hm_s = xpool.tile([M, D], F32)
                nc.scalar.copy(hm_s[:], fin_ps[:, D : 2 * D])
                outf = xpool.tile([M, D], F32)
                nc.vector.scalar_tensor_tensor(
                    outf[:],
                    fin_ps[:, 0:D],
                    rz_s[:],
                    hm_s[:],
                    mybir.AluOpType.mult,
                    mybir.AluOpType.add,
                )
                nc.sync.dma_start(out[b], outf[:])

    nc.finalize()
    return nc


def _host_precompute(curr_proc_batch, batch_idxes, feat_ope, feat_mac, W_ope, W_mac,
                     alpha_ope, alpha_mac):
    bi = np.asarray(batch_idxes)
    if np.array_equal(bi, np.arange(B)):
        proc = np.ascontiguousarray(curr_proc_batch, dtype=np.float32)
    else:
        proc = np.ascontiguousarray(curr_proc_batch[bi.astype(np.int64)],
                                    dtype=np.float32)

    fo = np.asarray(feat_ope, dtype=np.float32)
    fm = np.asarray(feat_mac, dtype=np.float32)
    Wo = np.asarray(W_ope, dtype=np.float64)
    Wm = np.asarray(W_mac, dtype=np.float64)

    v_ope = Wo.T @ np.asarray(alpha_ope, dtype=np.float64)
    v_mac = Wm.T @ np.asarray(alpha_mac, dtype=np.float64)
    a_ope = fo.astype(np.float64) @ v_ope          # [B, O]
    a_mac = fm.astype(np.float64) @ v_mac          # [B, M]

    p1 = np.exp(0.8 * a_ope).astype(np.float32)
    p2 = np.exp(0.2 * a_ope).astype(np.float32)
    q = np.exp(a_mac).astype(np.float32)
    q2 = np.exp(0.2 * a_mac).astype(np.float32)

    featA = np.empty((B, O, D + 1), np.float32)
    featA[:, :, :D] = fo * p2[:, :, None]
    featA[:, :, D] = p2
    # pre-tile to [b, c, p, j, d] and byte-pack featA + u8 proc into one blob
    featA_t = np.ascontiguousarray(
        featA.reshape(B, NCH, CH, P, D + 1).transpose(0, 1, 3, 2, 4))
    proc_t = np.ascontiguousarray(
        proc.astype(np.uint8).reshape(B, NCH, CH, P, M).transpose(0, 1, 3, 2, 4))
    fa_bytes = featA_t.view(np.uint8).reshape(B, NCH, P, CH * (D + 1) * 4)
    pr_bytes = proc_t.reshape(B, NCH, P, CH * M)
    blob = np.concatenate([fa_bytes, pr_bytes], axis=-1)   # [B, NCH, P, BLOBW]

    # rowsA strips, one per batch: [p1 (NCH*P) | block-diag q | q2x4 (row 0)]
    P1S, QS = NCH * P, CH * M
    STRW = P1S + 2 * QS
    rowsA = np.zeros((B, CH, STRW), np.float32)
    for g in range(B):
        # p1_4[j, c*P + p] = p1[g, (c*CH + j)*P + p]
        rowsA[g, :, :P1S] = (
            p1[g].reshape(NCH, CH, P).transpose(1, 0, 2).reshape(CH, P1S))
        for j in range(CH):
            rowsA[g, j, P1S + j * M : P1S + (j + 1) * M] = q[g]
        rowsA[g, 0, P1S + QS : P1S + 2 * QS] = np.tile(q2[g], CH)

    q2r_arr = np.tile(q2, (1, CH))                 # [B, CH*M]
    fmT = np.ascontiguousarray(fm.transpose(0, 2, 1))
    wot = np.ascontiguousarray(Wo.T).astype(np.float32)
    wmt = np.ascontiguousarray(Wm.T).astype(np.float32)
    return blob, rowsA, q2r_arr, fmT, wot, wmt


def kernel(curr_proc_batch, batch_idxes, feat_ope, feat_mac, W_ope, W_mac,
           alpha_ope, alpha_mac, _run_kwargs=None):
    blob, rowsA, q2r_arr, fmT, wot, wmt = _host_precompute(
        curr_proc_batch, batch_idxes, feat_ope, feat_mac, W_ope, W_mac,
        alpha_ope, alpha_mac)

    if "nc" not in _CACHE:
        _CACHE["nc"] = _build_nc()
    nc = _CACHE["nc"]

    in_maps = []
    for c in range(NCORES):
        s = slice(c * BPC, (c + 1) * BPC)
        in_maps.append({
            "blob": blob[s],
            "rowsA": rowsA[s],
            "q2r": q2r_arr[s],
            "fmT": fmT[s],
            "wot": wot,
            "wmt": wmt,
        })

    kw = _run_kwargs or {}
    res = run_bass_kernel_spmd(nc, in_maps, list(range(NCORES)), **kw)
    _CACHE["last_results"] = res
    out = np.concatenate([res.results[c]["out"] for c in range(NCORES)], axis=0)
    return out
